# revision 33
# baseline (speedup 1.0000x reference)
"""Trainium2 Bass kernel for nn_MsaHmmLayer: log-space HMM forward/backward
state posteriors + per-sequence log-likelihoods.

Strategy: linear-domain scaled forward/backward. The per-step logsumexp over
the 128 transition states becomes a 128x128 matvec on the PE (batched over 16
sequences per core), with emission multiplies on the DVE. States and
transition weights are fp16 (fp32 weights would re-stream the full 128x128
stationary matrix on every matmul); data-dependent rescaling of both the
forward and backward states every 128 steps, plus folded power-of-two shifts,
keeps everything inside fp16 normal range. Posteriors are normalized per
epoch with products of the rescale factors folded into the final Ln
activation's per-partition scale; all epoch bookkeeping telescopes so only
the raw colsum reciprocals are tracked.

Sharding: model m -> cores 2m, 2m+1; each core owns 16 of the 32 sequences of
one model and runs a completely independent fwd/bwd recursion (parameters
replicated host-side). Output shards are concatenated on the host.
"""
import sys

sys.path.insert(0, "/opt/trn_rl_repo")

import numpy as np

import concourse.bass as bass
import concourse.mybir as mybir
import concourse.tile as tile
from concourse.bass_utils import run_bass_kernel_spmd
from concourse.vector_clock import ScopedClock

F32 = mybir.dt.float32
F16 = mybir.dt.float16

NUM_MODEL, B, L, S, Q = 4, 32, 1024, 26, 128
NSEQ = 16           # sequences per core
RESCALE = 128
NEP = L // RESCALE  # rescale epochs
SH = 128.0          # per-rescale shift, folded into the broadcast ones-row
                    # (sized so z = psi*btilde stays in fp16 normal range)
SH_INIT = 1024.0    # init shift for both directions
N_CORES = 8

MAX_WAITS = 1  # walrus setupSyncWait limit per instruction (empirical)


class SplitDrainTileContext(tile.TileContext):
    """Walrus rejects instructions carrying more than one sync wait. After
    Tile scheduling, walk every basic block and move excess waits onto
    same-engine nops inserted immediately before the offending instruction
    (engines execute in order, so the waits still gate it)."""

    _waitnop_id = 0

    def _split_excess_waits(self):
        nc = self.nc
        for bb in nc.m.functions[0].blocks:
            insts = bb.instructions
            out = []
            changed = False
            for inst in insts:
                si = inst.sync_info
                if si is not None and si.on_wait and len(si.on_wait) > MAX_WAITS:
                    waits = list(si.on_wait)
                    for i in range(MAX_WAITS, len(waits), MAX_WAITS):
                        SplitDrainTileContext._waitnop_id += 1
                        nop = mybir.InstNoOp(
                            name=f"waitnop-{SplitDrainTileContext._waitnop_id}",
                            ins=[], outs=[],
                        )
                        nop.engine = inst.engine
                        nop.sync_info = mybir.SyncInfo(
                            on_wait=waits[i : i + MAX_WAITS], on_update=[]
                        )
                        out.append(nop)
                    si.on_wait = waits[:MAX_WAITS]
                    changed = True
                out.append(inst)
            if changed:
                bb.instructions = out

    _ldw_id = 0

    def _prefetch_ldweights(self):
        """Split each recursion matmul (fp16, full 128x128 weights) into an
        explicit InstLdweights (no sync wait -> streams immediately after the
        previous PE op) + a weight-stripped InstMatmult that carries the rhs
        wait. The constant transition matrices then load DURING the wait on
        the DVE-produced rhs instead of after it."""
        for bb in self.nc.m.functions[0].blocks:
            insts = bb.instructions
            out = []
            changed = False
            for inst in insts:
                if (type(inst).__name__ == "InstMatmult"
                        and len(inst.ins) == 2
                        and not inst.is_transpose):
                    wap = inst.ins[1]
                    ap = getattr(wap, "ap", None)
                    if (ap is not None and len(ap) == 2
                            and ap[0][1] == 128 and ap[1][1] == 128
                            and wap.dtype == mybir.dt.float16):
                        SplitDrainTileContext._ldw_id += 1
                        ldw = mybir.InstLdweights(
                            name=f"preldw-{SplitDrainTileContext._ldw_id}",
                            ins=[wap], outs=[],
                        )
                        ldw.engine = inst.engine
                        out.append(ldw)
                        inst.ins = [inst.ins[0]]
                        changed = True
                out.append(inst)
            if changed:
                bb.instructions = out

    def _drain_and_barrier(self, tick_clock, wait_clock):
        nc = self.nc
        collector = nc.sync.nop()
        wait_clock.add_sem_waits(
            collector.ins, ScopedClock({None: tick_clock.global_clock})
        )
        si = collector.ins.sync_info
        waits = list(si.on_wait) if si and si.on_wait else []
        if si is not None and len(waits) > 1:
            si.on_wait = waits[:1]
            for i in range(1, len(waits)):
                nxt = nc.sync.nop()
                nxt.ins.sync_info = mybir.SyncInfo(
                    on_wait=waits[i : i + 1], on_update=[]
                )
        nc.sync.drain()
        nc.all_engine_barrier()
        popped = nc._tile_sem_poison_stack.pop()
        assert popped is self._sem_poison
        nc.clear_and_free_semaphores(list(self.sems.allocated().values()))
        nc.all_engine_barrier()
        if getattr(self, "split_waits", True):
            self._split_excess_waits()


def build_program(length=L, split_waits=True):
    """Emit the per-core Bass program (identical across cores; SPMD)."""
    nep = max(1, length // RESCALE)
    resc = min(RESCALE, length)

    nc = bass.Bass("TRN2", target_bir_lowering=False, debug=False,
                   num_devices=N_CORES)

    obs_d = nc.dram_tensor("obs", [NSEQ, length, S], F32, kind="ExternalInput")
    af_d = nc.dram_tensor("afwd", [Q, Q], F16, kind="ExternalInput")
    ab_d = nc.dram_tensor("abwd", [Q, Q], F16, kind="ExternalInput")
    b2t_d = nc.dram_tensor("b2t", [S, Q], F32, kind="ExternalInput")
    pi_d = nc.dram_tensor("pi", [Q, 1], F32, kind="ExternalInput")
    onc_d = nc.dram_tensor("onesc", [Q, 1], F16, kind="ExternalInput")
    onrh_d = nc.dram_tensor("onesrh", [1, Q], F16, kind="ExternalInput")   # value SH
    onrf_d = nc.dram_tensor("onesrf", [1, Q], F32, kind="ExternalInput")   # value 1/SH_INIT
    idn_d = nc.dram_tensor("ident", [Q, Q], F32, kind="ExternalInput")
    idn16_d = nc.dram_tensor("ident16", [Q, Q], F16, kind="ExternalInput")

    post_d = nc.dram_tensor("post", [NSEQ, length, Q], F32, kind="ExternalOutput")
    rcv_d = nc.dram_tensor("rcv", [1, nep * NSEQ], F32, kind="ExternalOutput")

    tc_ctx = SplitDrainTileContext(nc)
    tc_ctx.split_waits = split_waits
    with tc_ctx as tc:
        with (
            tc.tile_pool(name="persist", bufs=1) as pp,
            tc.tile_pool(name="ldpool", bufs=2) as lp,
            tc.tile_pool(name="small", bufs=2) as sp,
            tc.tile_pool(name="stage", bufs=3) as stp,
            tc.tile_pool(name="ppf", bufs=2, space="PSUM") as ppf,
            tc.tile_pool(name="ppb", bufs=2, space="PSUM") as ppb,
            tc.tile_pool(name="prs", bufs=1, space="PSUM") as prs,
            tc.tile_pool(name="pbig", bufs=3, space="PSUM") as pbig,
        ):
            # ---- persistent SBUF ----
            # AB: fp16, [alpha-hat | b-tilde] halves, each seq-major [s*len + t]
            AB = pp.tile([Q, 2 * NSEQ * length], F16, name="AB")
            EB = pp.tile([Q, NSEQ * length], F32, name="EB")
            afw = pp.tile([Q, Q], F16, name="afw")
            abw = pp.tile([Q, Q], F16, name="abw")
            b2t = pp.tile([S, Q], F32, name="b2t")
            piv = pp.tile([Q, 1], F32, name="piv")
            onc = pp.tile([Q, 1], F16, name="onc")
            onrh = pp.tile([1, Q], F16, name="onrh")
            onrf = pp.tile([1, Q], F32, name="onrf")
            idn = pp.tile([Q, Q], F32, name="idn")
            idn16 = pp.tile([Q, Q], F16, name="idn16")
            rcst = pp.tile([1, nep * NSEQ], F32, name="rcst")
            cbst = pp.tile([1, nep * NSEQ], F32, name="cbst")
            ksuf = pp.tile([1, nep * NSEQ], F32, name="ksuf")
            kb = pp.tile([Q, nep * NSEQ], F32, name="kb")

            ab4 = AB[:].rearrange("p (h s t) -> p h s t", h=2, t=length)
            eb3 = EB[:].rearrange("p (s t) -> p s t", t=length)

            nc.sync.dma_start(afw[:], af_d.ap()[:])
            nc.sync.dma_start(abw[:], ab_d.ap()[:])
            nc.sync.dma_start(b2t[:], b2t_d.ap()[:])
            nc.sync.dma_start(piv[:], pi_d.ap()[:])
            nc.sync.dma_start(onc[:], onc_d.ap()[:])
            nc.sync.dma_start(onrh[:], onrh_d.ap()[:])
            nc.sync.dma_start(onrf[:], onrf_d.ap()[:])
            nc.sync.dma_start(idn[:], idn_d.ap()[:])
            nc.sync.dma_start(idn16[:], idn16_d.ap()[:])

            # ---- prep: emissions EB[q, s, t] = 2 * sum_s' B[q,s'] obs[s,t,s']
            # Per-(seq, 128-step chunk), waves ordered ends-first so both the
            # forward (t=0) and backward (t=L-1) recursions can start while
            # the middle chunks are still being produced.
            OBL = pp.tile([Q, NSEQ * (length // Q) * S], F32, name="OBL")
            nsub = length // Q
            for sq in range(NSEQ):
                src = obs_d.ap()[sq].rearrange("(u p) s -> p u s", p=Q)
                dst = OBL[:, sq * nsub * S : (sq + 1) * nsub * S]
                nc.sync.dma_start(dst.rearrange("p (u s) -> p u s", s=S), src)
            nchunk = 256 if length >= 256 else length
            ncs = nchunk // Q
            nch = length // nchunk
            corder = []
            lo, hi = 0, nch - 1
            while lo <= hi:
                if hi > lo:
                    corder.append(hi)
                corder.append(lo)
                lo, hi = lo + 1, hi - 1
            for c in corder:
                for sq in range(NSEQ):
                    ops = pbig.tile([S, nchunk], F32, tag="big")
                    for u in range(ncs):
                        k = sq * nsub + c * ncs + u
                        nc.tensor.transpose(
                            ops[:, u * Q : (u + 1) * Q],
                            OBL[:, k * S : (k + 1) * S],
                            idn[:],
                        )
                    otr = lp.tile([S, nchunk], F32, tag="otr")
                    nc.vector.tensor_copy(otr[:], ops[:])
                    eps = pbig.tile([Q, nchunk], F32, tag="big")
                    nc.tensor.matmul(eps[:], b2t[:], otr[:])
                    nc.scalar.copy(
                        eb3[:, sq, c * nchunk : (c + 1) * nchunk], eps[:]
                    )

            # ---- init (shifted by SH_INIT; piv = SH_INIT*pi host-side)
            # forward state lives in small ping tiles; AB[0] stores the
            # PRE-emission psums (psi) so the posterior z = psi * btilde
            # needs no division by e.
            H = NSEQ // 2  # seq-group size: 2 chains per direction
            af = sp.tile([Q, NSEQ], F16, tag="af", bufs=3)
            nc.vector.tensor_scalar_mul(af[:], eb3[:, :, 0], piv[:])
            nc.vector.memset(ab4[:, 0, :, 0], 1.0)
            nc.vector.tensor_scalar_mul(ab4[:, 0, :, 0], ab4[:, 0, :, 0], piv[:])
            nc.vector.tensor_scalar_mul(ab4[:, 1, :, length - 1],
                                        eb3[:, :, length - 1], float(SH_INIT))

            # ---- recursion ----
            # 4 independent dependency chains (2 seq-halves x fwd/bwd) hide
            # the per-chain matmul->mul->matmul latency. psF psums for PSW
            # consecutive steps share one PSUM tile so the psi stores batch
            # into one ACT copy per PSW steps.
            PSW = 4
            sb = None
            psFbig = None
            for w in range(1, length):
                tf = w
                tb = length - 1 - w
                k = (w - 1) % PSW
                if k == 0:
                    psFbig = ppf.tile([Q, PSW * NSEQ], F32, tag="pf")
                psF = psFbig[:, k * NSEQ : (k + 1) * NSEQ]
                nc.tensor.matmul(psF[:], afw[:], af[:])
                psB = ppb.tile([Q, NSEQ], F32, tag="pb", bufs=2)
                rhsB = sb[:] if sb is not None else ab4[:, 1, :, tb + 1]
                sb = None
                nc.tensor.matmul(psB[:], abw[:], rhsB)
                # bwd mul first: it eats the DVE inter-op restart bubble in the
                # fwd chain's shadow, so the latency-critical fwd mul runs in
                # the cheap back-to-back slot.
                nc.vector.tensor_mul(ab4[:, 1, :, tb], psB[:], eb3[:, :, tb])
                af = sp.tile([Q, NSEQ], F16, tag="af", bufs=3)
                nc.vector.tensor_mul(af[:], psF[:], eb3[:, :, tf])
                if k == PSW - 1 or w == length - 1:
                    cnt = k + 1
                    t0 = w - k
                    src = psFbig[:, 0 : cnt * NSEQ].rearrange(
                        "p (k s) -> p s k", s=NSEQ
                    )
                    nc.scalar.copy(ab4[:, 0, :, t0 : t0 + cnt], src)
                if tf % resc == resc - 1:
                    ep = tf // resc
                    cps = prs.tile([1, NSEQ], F32, tag="rs")
                    nc.tensor.matmul(cps[:], onc[:], af[:])
                    rcs = sp.tile([1, NSEQ], F16, tag="rc")
                    with nc.allow_low_precision(reason="fp16 rescale factor is applied and logged identically"):
                        nc.vector.reciprocal(rcs[:], cps[:])
                    nc.scalar.copy(rcst[:, ep * NSEQ : (ep + 1) * NSEQ], rcs[:])
                    if tf != length - 1:
                        bcp = prs.tile([Q, NSEQ], F32, tag="rs")
                        nc.tensor.matmul(bcp[:], onrh[:], rcs[:])
                        sa = sp.tile([Q, NSEQ], F16, tag="af", bufs=3)
                        nc.vector.tensor_mul(sa[:], af[:], bcp[:])
                        af = sa
                if tb % resc == 0 and tb > 0:
                    cpb = prs.tile([1, NSEQ], F32, tag="rs")
                    nc.tensor.matmul(cpb[:], onc[:], ab4[:, 1, :, tb])
                    rp = tb // resc
                    nc.scalar.copy(cbst[:, rp * NSEQ : (rp + 1) * NSEQ], cpb[:])
                    rcb = sp.tile([1, NSEQ], F16, tag="rc")
                    with nc.allow_low_precision(reason="fp16 rescale factor is applied and logged identically"):
                        nc.vector.reciprocal(rcb[:], cpb[:])
                    bcb = prs.tile([Q, NSEQ], F32, tag="rs")
                    nc.tensor.matmul(bcb[:], onrh[:], rcb[:])
                    sb = sp.tile([Q, NSEQ], F16, tag="sa")
                    nc.vector.tensor_mul(sb[:], ab4[:, 1, :, tb], bcb[:])

            # ---- z = psi * btilde into EB, chunked per (epoch, seq) and
            # ordered middle-out: those chunks' inputs complete while the
            # recursion is still running, so they overlap it.
            zorder = []
            lo, hi = (nep - 1) // 2, (nep + 2 - 1) // 2
            if nep == 1:
                zorder = [0]
            else:
                while lo >= 0:
                    zorder.append(lo)
                    if hi < nep:
                        zorder.append(hi)
                    lo, hi = lo - 1, hi + 1
            for ep in zorder:
                for sq in range(NSEQ):
                    r0 = ep * resc
                    nc.gpsimd.tensor_mul(ab4[:, 0, sq, r0 : r0 + resc],
                                         ab4[:, 0, sq, r0 : r0 + resc],
                                         ab4[:, 1, sq, r0 : r0 + resc])

            # ---- epoch normalizers:
            # P[nep-1] = rca[nep-1];  P[ep] = rca[ep] * cb[ep+1] * P[ep+1]
            # kb = broadcast(P / SH_INIT) over partitions
            nc.vector.tensor_copy(ksuf[:, (nep - 1) * NSEQ : nep * NSEQ],
                                  rcst[:, (nep - 1) * NSEQ : nep * NSEQ])
            for ep in range(nep - 2, -1, -1):
                nc.vector.tensor_mul(
                    ksuf[:, ep * NSEQ : (ep + 1) * NSEQ],
                    rcst[:, ep * NSEQ : (ep + 1) * NSEQ],
                    cbst[:, (ep + 1) * NSEQ : (ep + 2) * NSEQ],
                )
                nc.vector.tensor_mul(
                    ksuf[:, ep * NSEQ : (ep + 1) * NSEQ],
                    ksuf[:, ep * NSEQ : (ep + 1) * NSEQ],
                    ksuf[:, (ep + 1) * NSEQ : (ep + 2) * NSEQ],
                )
            kbp = prs.tile([Q, nep * NSEQ], F32, tag="rs")
            nc.tensor.matmul(kbp[:], onrf[:], ksuf[:])
            nc.scalar.copy(kb[:], kbp[:])

            # ---- transpose + log + store ----
            tchunk = min(resc, length)
            for sq in range(NSEQ):
                for ep in range(length // tchunk):
                    tp = pbig.tile([Q, tchunk], F16, tag="big")
                    for u in range(tchunk // Q):
                        t0 = ep * tchunk + u * Q
                        nc.tensor.transpose(
                            tp[:, u * Q : (u + 1) * Q],
                            ab4[:, 0, sq, t0 : t0 + Q],
                            idn16[:],
                        )
                    st = stp.tile([Q, tchunk], F32, tag="st")
                    epp = min(ep, nep - 1)
                    nc.scalar.activation(
                        st[:], tp[:], mybir.ActivationFunctionType.Ln,
                        scale=kb[:, epp * NSEQ + sq : epp * NSEQ + sq + 1],
                    )
                    dst = post_d.ap()[sq, ep * tchunk : (ep + 1) * tchunk, :]
                    nc.sync.dma_start(
                        dst.rearrange("(u p) q -> p u q", p=Q),
                        st[:].rearrange("p (u q) -> p u q", q=Q),
                    )

            nc.sync.dma_start(rcv_d.ap()[:], rcst[:])

    return nc


def _host_prep(transition_logits, init_logits, emission_logits):
    def softmax(v, ax):
        v = v - v.max(axis=ax, keepdims=True)
        ev = np.exp(v)
        return ev / ev.sum(axis=ax, keepdims=True)

    A = softmax(transition_logits.astype(np.float64), 2)
    pi = softmax(init_logits.astype(np.float64), 1)
    Bm = softmax(emission_logits.astype(np.float64), 2)
    return (A.astype(np.float32), pi.astype(np.float32),
            (2.0 * Bm).astype(np.float32))


_CACHED = {}
_last_in_maps = None


def kernel(inputs, transition_logits, init_logits, emission_logits):
    x = np.ascontiguousarray(np.asarray(inputs, dtype=np.float32))
    A, pi, B2 = _host_prep(np.asarray(transition_logits),
                           np.asarray(init_logits),
                           np.asarray(emission_logits))

    if "nc" not in _CACHED:
        _CACHED["nc"] = build_program(L)
    nc = _CACHED["nc"]

    ident = np.eye(Q, dtype=np.float32)
    onesc = np.ones((Q, 1), np.float16)
    onesrh = np.full((1, Q), SH, np.float16)
    onesrf = np.full((1, Q), 1.0 / SH_INIT, np.float32)

    in_maps = []
    for core in range(N_CORES):
        m, h = core // 2, core % 2
        in_maps.append({
            "obs": np.ascontiguousarray(x[m, NSEQ * h : NSEQ * (h + 1)]),
            "afwd": np.ascontiguousarray(A[m].astype(np.float16)),
            "abwd": np.ascontiguousarray(A[m].T.astype(np.float16)),
            "b2t": np.ascontiguousarray(B2[m].T),
            "pi": np.ascontiguousarray((SH_INIT * pi[m])[:, None].astype(np.float32)),
            "onesc": onesc,
            "onesrh": onesrh,
            "onesrf": onesrf,
            "ident": ident,
            "ident16": ident.astype(np.float16),
        })

    global _last_in_maps
    _last_in_maps = in_maps
    res = run_bass_kernel_spmd(nc, in_maps, list(range(N_CORES)))

    posterior = np.empty((NUM_MODEL, B, L, Q), np.float32)
    loglik = np.empty((NUM_MODEL, B), np.float32)
    for core in range(N_CORES):
        m, h = core // 2, core % 2
        posterior[m, NSEQ * h : NSEQ * (h + 1)] = res.results[core]["post"]
        rc = res.results[core]["rcv"].reshape(NEP, NSEQ).astype(np.float64)
        ll = (-np.log(rc).sum(axis=0) - (NEP - 1) * np.log(SH)
              - np.log(SH_INIT) - L * np.log(2.0))
        loglik[m, NSEQ * h : NSEQ * (h + 1)] = ll.astype(np.float32)
    return posterior, loglik


# revision 35
# speedup vs baseline: 1.0262x; 1.0262x over previous
"""Trainium2 Bass kernel for nn_MsaHmmLayer: log-space HMM forward/backward
state posteriors + per-sequence log-likelihoods.

Strategy: linear-domain scaled forward/backward. The per-step logsumexp over
the 128 transition states becomes a 128x128 matvec on the PE (batched over 16
sequences per core), with emission multiplies on the DVE. States and
transition weights are fp16 (fp32 weights would re-stream the full 128x128
stationary matrix on every matmul); data-dependent rescaling of both the
forward and backward states every 128 steps, plus folded power-of-two shifts,
keeps everything inside fp16 normal range. Posteriors are normalized per
epoch with products of the rescale factors folded into the final Ln
activation's per-partition scale; all epoch bookkeeping telescopes so only
the raw colsum reciprocals are tracked.

Sharding: model m -> cores 2m, 2m+1; each core owns 16 of the 32 sequences of
one model and runs a completely independent fwd/bwd recursion (parameters
replicated host-side). Output shards are concatenated on the host.
"""
import sys

sys.path.insert(0, "/opt/trn_rl_repo")

import numpy as np

import concourse.bass as bass
import concourse.mybir as mybir
import concourse.tile as tile
from concourse.bass_utils import run_bass_kernel_spmd
from concourse.vector_clock import ScopedClock

F32 = mybir.dt.float32
F16 = mybir.dt.float16

NUM_MODEL, B, L, S, Q = 4, 32, 1024, 26, 128
NSEQ = 16           # sequences per core
RESCALE = 128
NEP = L // RESCALE  # rescale epochs
SH = 128.0          # per-rescale shift, folded into the broadcast ones-row
                    # (sized so z = psi*btilde stays in fp16 normal range)
SH_INIT = 1024.0    # init shift for both directions
N_CORES = 8

MAX_WAITS = 1  # walrus setupSyncWait limit per instruction (empirical)


class SplitDrainTileContext(tile.TileContext):
    """Walrus rejects instructions carrying more than one sync wait. After
    Tile scheduling, walk every basic block and move excess waits onto
    same-engine nops inserted immediately before the offending instruction
    (engines execute in order, so the waits still gate it)."""

    _waitnop_id = 0

    def _split_excess_waits(self):
        nc = self.nc
        for bb in nc.m.functions[0].blocks:
            insts = bb.instructions
            out = []
            changed = False
            for inst in insts:
                si = inst.sync_info
                if si is not None and si.on_wait and len(si.on_wait) > MAX_WAITS:
                    waits = list(si.on_wait)
                    for i in range(MAX_WAITS, len(waits), MAX_WAITS):
                        SplitDrainTileContext._waitnop_id += 1
                        nop = mybir.InstNoOp(
                            name=f"waitnop-{SplitDrainTileContext._waitnop_id}",
                            ins=[], outs=[],
                        )
                        nop.engine = inst.engine
                        nop.sync_info = mybir.SyncInfo(
                            on_wait=waits[i : i + MAX_WAITS], on_update=[]
                        )
                        out.append(nop)
                    si.on_wait = waits[:MAX_WAITS]
                    changed = True
                out.append(inst)
            if changed:
                bb.instructions = out

    _ldw_id = 0

    def _prefetch_ldweights(self):
        """Split each recursion matmul (fp16, full 128x128 weights) into an
        explicit InstLdweights (no sync wait -> streams immediately after the
        previous PE op) + a weight-stripped InstMatmult that carries the rhs
        wait. The constant transition matrices then load DURING the wait on
        the DVE-produced rhs instead of after it."""
        for bb in self.nc.m.functions[0].blocks:
            insts = bb.instructions
            out = []
            changed = False
            for inst in insts:
                if (type(inst).__name__ == "InstMatmult"
                        and len(inst.ins) == 2
                        and not inst.is_transpose):
                    wap = inst.ins[1]
                    ap = getattr(wap, "ap", None)
                    if (ap is not None and len(ap) == 2
                            and ap[0][1] == 128 and ap[1][1] == 128
                            and wap.dtype == mybir.dt.float16):
                        SplitDrainTileContext._ldw_id += 1
                        ldw = mybir.InstLdweights(
                            name=f"preldw-{SplitDrainTileContext._ldw_id}",
                            ins=[wap], outs=[],
                        )
                        ldw.engine = inst.engine
                        out.append(ldw)
                        inst.ins = [inst.ins[0]]
                        changed = True
                out.append(inst)
            if changed:
                bb.instructions = out

    def _drain_and_barrier(self, tick_clock, wait_clock):
        nc = self.nc
        collector = nc.sync.nop()
        wait_clock.add_sem_waits(
            collector.ins, ScopedClock({None: tick_clock.global_clock})
        )
        si = collector.ins.sync_info
        waits = list(si.on_wait) if si and si.on_wait else []
        if si is not None and len(waits) > 1:
            si.on_wait = waits[:1]
            for i in range(1, len(waits)):
                nxt = nc.sync.nop()
                nxt.ins.sync_info = mybir.SyncInfo(
                    on_wait=waits[i : i + 1], on_update=[]
                )
        nc.sync.drain()
        nc.all_engine_barrier()
        popped = nc._tile_sem_poison_stack.pop()
        assert popped is self._sem_poison
        nc.clear_and_free_semaphores(list(self.sems.allocated().values()))
        nc.all_engine_barrier()
        if getattr(self, "split_waits", True):
            self._split_excess_waits()


def build_program(length=L, split_waits=True):
    """Emit the per-core Bass program (identical across cores; SPMD)."""
    nep = max(1, length // RESCALE)
    resc = min(RESCALE, length)

    nc = bass.Bass("TRN2", target_bir_lowering=False, debug=False,
                   num_devices=N_CORES)

    obs_d = nc.dram_tensor("obs", [NSEQ, length, S], F32, kind="ExternalInput")
    af_d = nc.dram_tensor("afwd", [Q, Q], F16, kind="ExternalInput")
    ab_d = nc.dram_tensor("abwd", [Q, Q], F16, kind="ExternalInput")
    b2t_d = nc.dram_tensor("b2t", [S, Q], F32, kind="ExternalInput")
    pi_d = nc.dram_tensor("pi", [Q, 1], F32, kind="ExternalInput")
    onc_d = nc.dram_tensor("onesc", [Q, 1], F16, kind="ExternalInput")
    onrh_d = nc.dram_tensor("onesrh", [1, Q], F16, kind="ExternalInput")   # value SH
    onrf_d = nc.dram_tensor("onesrf", [1, Q], F32, kind="ExternalInput")   # value 1/SH_INIT
    idn_d = nc.dram_tensor("ident", [Q, Q], F32, kind="ExternalInput")
    idn16_d = nc.dram_tensor("ident16", [Q, Q], F16, kind="ExternalInput")

    post_d = nc.dram_tensor("post", [NSEQ, length, Q], F32, kind="ExternalOutput")
    rcv_d = nc.dram_tensor("rcv", [1, nep * NSEQ], F32, kind="ExternalOutput")

    tc_ctx = SplitDrainTileContext(nc)
    tc_ctx.split_waits = split_waits
    with tc_ctx as tc:
        with (
            tc.tile_pool(name="persist", bufs=1) as pp,
            tc.tile_pool(name="ldpool", bufs=2) as lp,
            tc.tile_pool(name="small", bufs=2) as sp,
            tc.tile_pool(name="stage", bufs=10) as stp,
            tc.tile_pool(name="ppf", bufs=2, space="PSUM") as ppf,
            tc.tile_pool(name="ppb", bufs=2, space="PSUM") as ppb,
            tc.tile_pool(name="prs", bufs=1, space="PSUM") as prs,
            tc.tile_pool(name="pbig", bufs=2, space="PSUM") as pbig,
        ):
            # ---- persistent SBUF ----
            # AB: fp16, [alpha-hat | b-tilde] halves, each seq-major [s*len + t]
            AB = pp.tile([Q, 2 * NSEQ * length], F16, name="AB")
            EB = pp.tile([Q, NSEQ * length], F32, name="EB")
            afw = pp.tile([Q, Q], F16, name="afw")
            abw = pp.tile([Q, Q], F16, name="abw")
            b2t = pp.tile([S, Q], F32, name="b2t")
            piv = pp.tile([Q, 1], F32, name="piv")
            onc = pp.tile([Q, 1], F16, name="onc")
            onrh = pp.tile([1, Q], F16, name="onrh")
            onrf = pp.tile([1, Q], F32, name="onrf")
            idn = pp.tile([Q, Q], F32, name="idn")
            idn16 = pp.tile([Q, Q], F16, name="idn16")
            rcst = pp.tile([1, nep * NSEQ], F32, name="rcst")
            cbst = pp.tile([1, nep * NSEQ], F32, name="cbst")
            ksuf = pp.tile([1, nep * NSEQ], F32, name="ksuf")
            kb = pp.tile([Q, nep * NSEQ], F32, name="kb")

            ab4 = AB[:].rearrange("p (h s t) -> p h s t", h=2, t=length)
            eb3 = EB[:].rearrange("p (s t) -> p s t", t=length)

            nc.sync.dma_start(afw[:], af_d.ap()[:])
            nc.sync.dma_start(abw[:], ab_d.ap()[:])
            nc.sync.dma_start(b2t[:], b2t_d.ap()[:])
            nc.sync.dma_start(piv[:], pi_d.ap()[:])
            nc.sync.dma_start(onc[:], onc_d.ap()[:])
            nc.sync.dma_start(onrh[:], onrh_d.ap()[:])
            nc.sync.dma_start(onrf[:], onrf_d.ap()[:])
            nc.sync.dma_start(idn[:], idn_d.ap()[:])
            nc.sync.dma_start(idn16[:], idn16_d.ap()[:])

            # ---- prep: emissions EB[q, s, t] = 2 * sum_s' B[q,s'] obs[s,t,s']
            # Per-(seq, 128-step chunk), waves ordered ends-first so both the
            # forward (t=0) and backward (t=L-1) recursions can start while
            # the middle chunks are still being produced.
            OBL = pp.tile([Q, NSEQ * (length // Q) * S], F32, name="OBL")
            nsub = length // Q
            for sq in range(NSEQ):
                src = obs_d.ap()[sq].rearrange("(u p) s -> p u s", p=Q)
                dst = OBL[:, sq * nsub * S : (sq + 1) * nsub * S]
                nc.sync.dma_start(dst.rearrange("p (u s) -> p u s", s=S), src)
            nchunk = 256 if length >= 256 else length
            ncs = nchunk // Q
            nch = length // nchunk
            corder = []
            lo, hi = 0, nch - 1
            while lo <= hi:
                if hi > lo:
                    corder.append(hi)
                corder.append(lo)
                lo, hi = lo + 1, hi - 1
            for c in corder:
                for sq in range(NSEQ):
                    ops = pbig.tile([S, nchunk], F32, tag="big")
                    for u in range(ncs):
                        k = sq * nsub + c * ncs + u
                        nc.tensor.transpose(
                            ops[:, u * Q : (u + 1) * Q],
                            OBL[:, k * S : (k + 1) * S],
                            idn[:],
                        )
                    otr = lp.tile([S, nchunk], F32, tag="otr")
                    nc.vector.tensor_copy(otr[:], ops[:])
                    eps = pbig.tile([Q, nchunk], F32, tag="big")
                    nc.tensor.matmul(eps[:], b2t[:], otr[:])
                    nc.scalar.copy(
                        eb3[:, sq, c * nchunk : (c + 1) * nchunk], eps[:]
                    )

            # ---- init (shifted by SH_INIT; piv = SH_INIT*pi host-side)
            # forward state lives in small ping tiles; AB[0] stores the
            # PRE-emission psums (psi) so the posterior z = psi * btilde
            # needs no division by e.
            H = NSEQ // 2  # seq-group size: 2 chains per direction
            af = sp.tile([Q, NSEQ], F16, tag="af", bufs=3)
            nc.vector.tensor_scalar_mul(af[:], eb3[:, :, 0], piv[:])
            nc.vector.memset(ab4[:, 0, :, 0], 1.0)
            nc.vector.tensor_scalar_mul(ab4[:, 0, :, 0], ab4[:, 0, :, 0], piv[:])
            nc.vector.tensor_scalar_mul(ab4[:, 1, :, length - 1],
                                        eb3[:, :, length - 1], float(SH_INIT))

            # ---- recursion ----
            # 4 independent dependency chains (2 seq-halves x fwd/bwd) hide
            # the per-chain matmul->mul->matmul latency. psF psums for PSW
            # consecutive steps share one PSUM tile so the psi stores batch
            # into one ACT copy per PSW steps.
            PSW = 4
            sb = None
            psFbig = None
            for w in range(1, length):
                tf = w
                tb = length - 1 - w
                k = (w - 1) % PSW
                if k == 0:
                    psFbig = ppf.tile([Q, PSW * NSEQ], F32, tag="pf")
                psF = psFbig[:, k * NSEQ : (k + 1) * NSEQ]
                nc.tensor.matmul(psF[:], afw[:], af[:])
                psB = ppb.tile([Q, NSEQ], F32, tag="pb", bufs=3)
                rhsB = sb[:] if sb is not None else ab4[:, 1, :, tb + 1]
                sb = None
                nc.tensor.matmul(psB[:], abw[:], rhsB)
                # bwd mul first: it eats the DVE inter-op restart bubble in the
                # fwd chain's shadow, so the latency-critical fwd mul runs in
                # the cheap back-to-back slot.
                nc.vector.tensor_mul(ab4[:, 1, :, tb], psB[:], eb3[:, :, tb])
                af = sp.tile([Q, NSEQ], F16, tag="af", bufs=3)
                nc.vector.tensor_mul(af[:], psF[:], eb3[:, :, tf])
                if k == PSW - 1 or w == length - 1:
                    cnt = k + 1
                    t0 = w - k
                    src = psFbig[:, 0 : cnt * NSEQ].rearrange(
                        "p (k s) -> p s k", s=NSEQ
                    )
                    nc.scalar.copy(ab4[:, 0, :, t0 : t0 + cnt], src)
                if tf % resc == resc - 1:
                    ep = tf // resc
                    cps = prs.tile([1, NSEQ], F32, tag="rs")
                    nc.tensor.matmul(cps[:], onc[:], af[:])
                    rcs = sp.tile([1, NSEQ], F16, tag="rc")
                    with nc.allow_low_precision(reason="fp16 rescale factor is applied and logged identically"):
                        nc.vector.reciprocal(rcs[:], cps[:])
                    nc.scalar.copy(rcst[:, ep * NSEQ : (ep + 1) * NSEQ], rcs[:])
                    if tf != length - 1:
                        bcp = prs.tile([Q, NSEQ], F32, tag="rs")
                        nc.tensor.matmul(bcp[:], onrh[:], rcs[:])
                        sa = sp.tile([Q, NSEQ], F16, tag="af", bufs=3)
                        nc.vector.tensor_mul(sa[:], af[:], bcp[:])
                        af = sa
                if tb % resc == 0 and tb > 0:
                    cpb = prs.tile([1, NSEQ], F32, tag="rs")
                    nc.tensor.matmul(cpb[:], onc[:], ab4[:, 1, :, tb])
                    rp = tb // resc
                    nc.scalar.copy(cbst[:, rp * NSEQ : (rp + 1) * NSEQ], cpb[:])
                    rcb = sp.tile([1, NSEQ], F16, tag="rc")
                    with nc.allow_low_precision(reason="fp16 rescale factor is applied and logged identically"):
                        nc.vector.reciprocal(rcb[:], cpb[:])
                    bcb = prs.tile([Q, NSEQ], F32, tag="rs")
                    nc.tensor.matmul(bcb[:], onrh[:], rcb[:])
                    sb = sp.tile([Q, NSEQ], F16, tag="sa")
                    nc.vector.tensor_mul(sb[:], ab4[:, 1, :, tb], bcb[:])

            # ---- z = psi * btilde into EB, chunked per (epoch, seq) and
            # ordered middle-out: those chunks' inputs complete while the
            # recursion is still running, so they overlap it.
            zorder = []
            lo, hi = (nep - 1) // 2, (nep + 2 - 1) // 2
            if nep == 1:
                zorder = [0]
            else:
                while lo >= 0:
                    zorder.append(lo)
                    if hi < nep:
                        zorder.append(hi)
                    lo, hi = lo - 1, hi + 1
            for ep in zorder:
                for sq in range(NSEQ):
                    r0 = ep * resc
                    nc.gpsimd.tensor_mul(ab4[:, 0, sq, r0 : r0 + resc],
                                         ab4[:, 0, sq, r0 : r0 + resc],
                                         ab4[:, 1, sq, r0 : r0 + resc])

            # ---- epoch normalizers:
            # P[nep-1] = rca[nep-1];  P[ep] = rca[ep] * cb[ep+1] * P[ep+1]
            # kb = broadcast(P / SH_INIT) over partitions
            nc.vector.tensor_copy(ksuf[:, (nep - 1) * NSEQ : nep * NSEQ],
                                  rcst[:, (nep - 1) * NSEQ : nep * NSEQ])
            for ep in range(nep - 2, -1, -1):
                nc.vector.tensor_mul(
                    ksuf[:, ep * NSEQ : (ep + 1) * NSEQ],
                    rcst[:, ep * NSEQ : (ep + 1) * NSEQ],
                    cbst[:, (ep + 1) * NSEQ : (ep + 2) * NSEQ],
                )
                nc.vector.tensor_mul(
                    ksuf[:, ep * NSEQ : (ep + 1) * NSEQ],
                    ksuf[:, ep * NSEQ : (ep + 1) * NSEQ],
                    ksuf[:, (ep + 1) * NSEQ : (ep + 2) * NSEQ],
                )
            kbp = prs.tile([Q, nep * NSEQ], F32, tag="rs")
            nc.tensor.matmul(kbp[:], onrf[:], ksuf[:])
            nc.scalar.copy(kb[:], kbp[:])

            # ---- transpose + log + store ----
            tchunk = min(resc, length)
            for sq in range(NSEQ):
                for ep in range(length // tchunk):
                    tp = pbig.tile([Q, tchunk], F16, tag="big")
                    for u in range(tchunk // Q):
                        t0 = ep * tchunk + u * Q
                        nc.tensor.transpose(
                            tp[:, u * Q : (u + 1) * Q],
                            ab4[:, 0, sq, t0 : t0 + Q],
                            idn16[:],
                        )
                    st = stp.tile([Q, tchunk], F32, tag="st")
                    epp = min(ep, nep - 1)
                    nc.scalar.activation(
                        st[:], tp[:], mybir.ActivationFunctionType.Ln,
                        scale=kb[:, epp * NSEQ + sq : epp * NSEQ + sq + 1],
                    )
                    dst = post_d.ap()[sq, ep * tchunk : (ep + 1) * tchunk, :]
                    nc.sync.dma_start(
                        dst.rearrange("(u p) q -> p u q", p=Q),
                        st[:].rearrange("p (u q) -> p u q", q=Q),
                    )

            nc.sync.dma_start(rcv_d.ap()[:], rcst[:])

    return nc


def _host_prep(transition_logits, init_logits, emission_logits):
    def softmax(v, ax):
        v = v - v.max(axis=ax, keepdims=True)
        ev = np.exp(v)
        return ev / ev.sum(axis=ax, keepdims=True)

    A = softmax(transition_logits.astype(np.float64), 2)
    pi = softmax(init_logits.astype(np.float64), 1)
    Bm = softmax(emission_logits.astype(np.float64), 2)
    return (A.astype(np.float32), pi.astype(np.float32),
            (2.0 * Bm).astype(np.float32))


_CACHED = {}
_last_in_maps = None


def kernel(inputs, transition_logits, init_logits, emission_logits):
    x = np.ascontiguousarray(np.asarray(inputs, dtype=np.float32))
    A, pi, B2 = _host_prep(np.asarray(transition_logits),
                           np.asarray(init_logits),
                           np.asarray(emission_logits))

    if "nc" not in _CACHED:
        _CACHED["nc"] = build_program(L)
    nc = _CACHED["nc"]

    ident = np.eye(Q, dtype=np.float32)
    onesc = np.ones((Q, 1), np.float16)
    onesrh = np.full((1, Q), SH, np.float16)
    onesrf = np.full((1, Q), 1.0 / SH_INIT, np.float32)

    in_maps = []
    for core in range(N_CORES):
        m, h = core // 2, core % 2
        in_maps.append({
            "obs": np.ascontiguousarray(x[m, NSEQ * h : NSEQ * (h + 1)]),
            "afwd": np.ascontiguousarray(A[m].astype(np.float16)),
            "abwd": np.ascontiguousarray(A[m].T.astype(np.float16)),
            "b2t": np.ascontiguousarray(B2[m].T),
            "pi": np.ascontiguousarray((SH_INIT * pi[m])[:, None].astype(np.float32)),
            "onesc": onesc,
            "onesrh": onesrh,
            "onesrf": onesrf,
            "ident": ident,
            "ident16": ident.astype(np.float16),
        })

    global _last_in_maps
    _last_in_maps = in_maps
    res = run_bass_kernel_spmd(nc, in_maps, list(range(N_CORES)))

    posterior = np.empty((NUM_MODEL, B, L, Q), np.float32)
    loglik = np.empty((NUM_MODEL, B), np.float32)
    for core in range(N_CORES):
        m, h = core // 2, core % 2
        posterior[m, NSEQ * h : NSEQ * (h + 1)] = res.results[core]["post"]
        rc = res.results[core]["rcv"].reshape(NEP, NSEQ).astype(np.float64)
        ll = (-np.log(rc).sum(axis=0) - (NEP - 1) * np.log(SH)
              - np.log(SH_INIT) - L * np.log(2.0))
        loglik[m, NSEQ * h : NSEQ * (h + 1)] = ll.astype(np.float32)
    return posterior, loglik


# revision 36
# speedup vs baseline: 1.0269x; 1.0007x over previous
"""Trainium2 Bass kernel for nn_MsaHmmLayer: log-space HMM forward/backward
state posteriors + per-sequence log-likelihoods.

Strategy: linear-domain scaled forward/backward. The per-step logsumexp over
the 128 transition states becomes a 128x128 matvec on the PE (batched over 16
sequences per core), with emission multiplies on the DVE. States and
transition weights are fp16 (fp32 weights would re-stream the full 128x128
stationary matrix on every matmul); data-dependent rescaling of both the
forward and backward states every 128 steps, plus folded power-of-two shifts,
keeps everything inside fp16 normal range. Posteriors are normalized per
epoch with products of the rescale factors folded into the final Ln
activation's per-partition scale; all epoch bookkeeping telescopes so only
the raw colsum reciprocals are tracked.

Sharding: model m -> cores 2m, 2m+1; each core owns 16 of the 32 sequences of
one model and runs a completely independent fwd/bwd recursion (parameters
replicated host-side). Output shards are concatenated on the host.
"""
import sys

sys.path.insert(0, "/opt/trn_rl_repo")

import numpy as np

import concourse.bass as bass
import concourse.mybir as mybir
import concourse.tile as tile
from concourse.bass_utils import run_bass_kernel_spmd
from concourse.vector_clock import ScopedClock

F32 = mybir.dt.float32
F16 = mybir.dt.float16

NUM_MODEL, B, L, S, Q = 4, 32, 1024, 26, 128
NSEQ = 16           # sequences per core
RESCALE = 128
NEP = L // RESCALE  # rescale epochs
SH = 128.0          # per-rescale shift, folded into the broadcast ones-row
                    # (sized so z = psi*btilde stays in fp16 normal range)
SH_INIT = 1024.0    # init shift for both directions
N_CORES = 8

MAX_WAITS = 1  # walrus setupSyncWait limit per instruction (empirical)


class SplitDrainTileContext(tile.TileContext):
    """Walrus rejects instructions carrying more than one sync wait. After
    Tile scheduling, walk every basic block and move excess waits onto
    same-engine nops inserted immediately before the offending instruction
    (engines execute in order, so the waits still gate it)."""

    _waitnop_id = 0

    def _split_excess_waits(self):
        nc = self.nc
        for bb in nc.m.functions[0].blocks:
            insts = bb.instructions
            out = []
            changed = False
            for inst in insts:
                si = inst.sync_info
                if si is not None and si.on_wait and len(si.on_wait) > MAX_WAITS:
                    waits = list(si.on_wait)
                    for i in range(MAX_WAITS, len(waits), MAX_WAITS):
                        SplitDrainTileContext._waitnop_id += 1
                        nop = mybir.InstNoOp(
                            name=f"waitnop-{SplitDrainTileContext._waitnop_id}",
                            ins=[], outs=[],
                        )
                        nop.engine = inst.engine
                        nop.sync_info = mybir.SyncInfo(
                            on_wait=waits[i : i + MAX_WAITS], on_update=[]
                        )
                        out.append(nop)
                    si.on_wait = waits[:MAX_WAITS]
                    changed = True
                out.append(inst)
            if changed:
                bb.instructions = out

    _ldw_id = 0

    def _prefetch_ldweights(self):
        """Split each recursion matmul (fp16, full 128x128 weights) into an
        explicit InstLdweights (no sync wait -> streams immediately after the
        previous PE op) + a weight-stripped InstMatmult that carries the rhs
        wait. The constant transition matrices then load DURING the wait on
        the DVE-produced rhs instead of after it."""
        for bb in self.nc.m.functions[0].blocks:
            insts = bb.instructions
            out = []
            changed = False
            for inst in insts:
                if (type(inst).__name__ == "InstMatmult"
                        and len(inst.ins) == 2
                        and not inst.is_transpose):
                    wap = inst.ins[1]
                    ap = getattr(wap, "ap", None)
                    if (ap is not None and len(ap) == 2
                            and ap[0][1] == 128 and ap[1][1] == 128
                            and wap.dtype == mybir.dt.float16):
                        SplitDrainTileContext._ldw_id += 1
                        ldw = mybir.InstLdweights(
                            name=f"preldw-{SplitDrainTileContext._ldw_id}",
                            ins=[wap], outs=[],
                        )
                        ldw.engine = inst.engine
                        out.append(ldw)
                        inst.ins = [inst.ins[0]]
                        changed = True
                out.append(inst)
            if changed:
                bb.instructions = out

    def _drain_and_barrier(self, tick_clock, wait_clock):
        nc = self.nc
        collector = nc.sync.nop()
        wait_clock.add_sem_waits(
            collector.ins, ScopedClock({None: tick_clock.global_clock})
        )
        si = collector.ins.sync_info
        waits = list(si.on_wait) if si and si.on_wait else []
        if si is not None and len(waits) > 1:
            si.on_wait = waits[:1]
            for i in range(1, len(waits)):
                nxt = nc.sync.nop()
                nxt.ins.sync_info = mybir.SyncInfo(
                    on_wait=waits[i : i + 1], on_update=[]
                )
        nc.sync.drain()
        nc.all_engine_barrier()
        popped = nc._tile_sem_poison_stack.pop()
        assert popped is self._sem_poison
        nc.clear_and_free_semaphores(list(self.sems.allocated().values()))
        nc.all_engine_barrier()
        if getattr(self, "split_waits", True):
            self._split_excess_waits()


def build_program(length=L, split_waits=True):
    """Emit the per-core Bass program (identical across cores; SPMD)."""
    nep = max(1, length // RESCALE)
    resc = min(RESCALE, length)

    nc = bass.Bass("TRN2", target_bir_lowering=False, debug=False,
                   num_devices=N_CORES)

    obs_d = nc.dram_tensor("obs", [NSEQ, length, S], F32, kind="ExternalInput")
    af_d = nc.dram_tensor("afwd", [Q, Q], F16, kind="ExternalInput")
    ab_d = nc.dram_tensor("abwd", [Q, Q], F16, kind="ExternalInput")
    b2t_d = nc.dram_tensor("b2t", [S, Q], F32, kind="ExternalInput")
    pi_d = nc.dram_tensor("pi", [Q, 1], F32, kind="ExternalInput")
    onc_d = nc.dram_tensor("onesc", [Q, 1], F16, kind="ExternalInput")
    onrh_d = nc.dram_tensor("onesrh", [1, Q], F16, kind="ExternalInput")   # value SH
    onrf_d = nc.dram_tensor("onesrf", [1, Q], F32, kind="ExternalInput")   # value 1/SH_INIT
    idn_d = nc.dram_tensor("ident", [Q, Q], F32, kind="ExternalInput")
    idn16_d = nc.dram_tensor("ident16", [Q, Q], F16, kind="ExternalInput")

    post_d = nc.dram_tensor("post", [NSEQ, length, Q], F32, kind="ExternalOutput")
    rcv_d = nc.dram_tensor("rcv", [1, nep * NSEQ], F32, kind="ExternalOutput")

    tc_ctx = SplitDrainTileContext(nc)
    tc_ctx.split_waits = split_waits
    with tc_ctx as tc:
        with (
            tc.tile_pool(name="persist", bufs=1) as pp,
            tc.tile_pool(name="ldpool", bufs=4) as lp,
            tc.tile_pool(name="small", bufs=2) as sp,
            tc.tile_pool(name="stage", bufs=10) as stp,
            tc.tile_pool(name="ppf", bufs=2, space="PSUM") as ppf,
            tc.tile_pool(name="ppb", bufs=2, space="PSUM") as ppb,
            tc.tile_pool(name="prs", bufs=1, space="PSUM") as prs,
            tc.tile_pool(name="pbig", bufs=2, space="PSUM") as pbig,
        ):
            # ---- persistent SBUF ----
            # AB: fp16, [alpha-hat | b-tilde] halves, each seq-major [s*len + t]
            AB = pp.tile([Q, 2 * NSEQ * length], F16, name="AB")
            EB = pp.tile([Q, NSEQ * length], F32, name="EB")
            afw = pp.tile([Q, Q], F16, name="afw")
            abw = pp.tile([Q, Q], F16, name="abw")
            b2t = pp.tile([S, Q], F32, name="b2t")
            piv = pp.tile([Q, 1], F32, name="piv")
            onc = pp.tile([Q, 1], F16, name="onc")
            onrh = pp.tile([1, Q], F16, name="onrh")
            onrf = pp.tile([1, Q], F32, name="onrf")
            idn = pp.tile([Q, Q], F32, name="idn")
            idn16 = pp.tile([Q, Q], F16, name="idn16")
            rcst = pp.tile([1, nep * NSEQ], F32, name="rcst")
            cbst = pp.tile([1, nep * NSEQ], F32, name="cbst")
            ksuf = pp.tile([1, nep * NSEQ], F32, name="ksuf")
            kb = pp.tile([Q, nep * NSEQ], F32, name="kb")

            ab4 = AB[:].rearrange("p (h s t) -> p h s t", h=2, t=length)
            eb3 = EB[:].rearrange("p (s t) -> p s t", t=length)

            nc.sync.dma_start(afw[:], af_d.ap()[:])
            nc.sync.dma_start(abw[:], ab_d.ap()[:])
            nc.sync.dma_start(b2t[:], b2t_d.ap()[:])
            nc.sync.dma_start(piv[:], pi_d.ap()[:])
            nc.sync.dma_start(onc[:], onc_d.ap()[:])
            nc.sync.dma_start(onrh[:], onrh_d.ap()[:])
            nc.sync.dma_start(onrf[:], onrf_d.ap()[:])
            nc.sync.dma_start(idn[:], idn_d.ap()[:])
            nc.sync.dma_start(idn16[:], idn16_d.ap()[:])

            # ---- prep: emissions EB[q, s, t] = 2 * sum_s' B[q,s'] obs[s,t,s']
            # Per-(seq, 128-step chunk), waves ordered ends-first so both the
            # forward (t=0) and backward (t=L-1) recursions can start while
            # the middle chunks are still being produced.
            OBL = pp.tile([Q, NSEQ * (length // Q) * S], F32, name="OBL")
            nsub = length // Q
            for sq in range(NSEQ):
                src = obs_d.ap()[sq].rearrange("(u p) s -> p u s", p=Q)
                dst = OBL[:, sq * nsub * S : (sq + 1) * nsub * S]
                nc.sync.dma_start(dst.rearrange("p (u s) -> p u s", s=S), src)
            nchunk = 256 if length >= 256 else length
            ncs = nchunk // Q
            nch = length // nchunk
            corder = []
            lo, hi = 0, nch - 1
            while lo <= hi:
                if hi > lo:
                    corder.append(hi)
                corder.append(lo)
                lo, hi = lo + 1, hi - 1
            for c in corder:
                for sq in range(NSEQ):
                    ops = pbig.tile([S, nchunk], F32, tag="big")
                    for u in range(ncs):
                        k = sq * nsub + c * ncs + u
                        nc.tensor.transpose(
                            ops[:, u * Q : (u + 1) * Q],
                            OBL[:, k * S : (k + 1) * S],
                            idn[:],
                        )
                    otr = lp.tile([S, nchunk], F32, tag="otr")
                    nc.vector.tensor_copy(otr[:], ops[:])
                    eps = pbig.tile([Q, nchunk], F32, tag="big")
                    nc.tensor.matmul(eps[:], b2t[:], otr[:])
                    nc.scalar.copy(
                        eb3[:, sq, c * nchunk : (c + 1) * nchunk], eps[:]
                    )

            # ---- init (shifted by SH_INIT; piv = SH_INIT*pi host-side)
            # forward state lives in small ping tiles; AB[0] stores the
            # PRE-emission psums (psi) so the posterior z = psi * btilde
            # needs no division by e.
            H = NSEQ // 2  # seq-group size: 2 chains per direction
            af = sp.tile([Q, NSEQ], F16, tag="af", bufs=3)
            nc.vector.tensor_scalar_mul(af[:], eb3[:, :, 0], piv[:])
            nc.vector.memset(ab4[:, 0, :, 0], 1.0)
            nc.vector.tensor_scalar_mul(ab4[:, 0, :, 0], ab4[:, 0, :, 0], piv[:])
            nc.vector.tensor_scalar_mul(ab4[:, 1, :, length - 1],
                                        eb3[:, :, length - 1], float(SH_INIT))

            # ---- recursion ----
            # 4 independent dependency chains (2 seq-halves x fwd/bwd) hide
            # the per-chain matmul->mul->matmul latency. psF psums for PSW
            # consecutive steps share one PSUM tile so the psi stores batch
            # into one ACT copy per PSW steps.
            PSW = 4
            sb = None
            psFbig = None
            for w in range(1, length):
                tf = w
                tb = length - 1 - w
                k = (w - 1) % PSW
                if k == 0:
                    psFbig = ppf.tile([Q, PSW * NSEQ], F32, tag="pf")
                psF = psFbig[:, k * NSEQ : (k + 1) * NSEQ]
                nc.tensor.matmul(psF[:], afw[:], af[:])
                psB = ppb.tile([Q, NSEQ], F32, tag="pb", bufs=3)
                rhsB = sb[:] if sb is not None else ab4[:, 1, :, tb + 1]
                sb = None
                nc.tensor.matmul(psB[:], abw[:], rhsB)
                # bwd mul first: it eats the DVE inter-op restart bubble in the
                # fwd chain's shadow, so the latency-critical fwd mul runs in
                # the cheap back-to-back slot.
                nc.vector.tensor_mul(ab4[:, 1, :, tb], psB[:], eb3[:, :, tb])
                af = sp.tile([Q, NSEQ], F16, tag="af", bufs=3)
                nc.vector.tensor_mul(af[:], psF[:], eb3[:, :, tf])
                if k == PSW - 1 or w == length - 1:
                    cnt = k + 1
                    t0 = w - k
                    src = psFbig[:, 0 : cnt * NSEQ].rearrange(
                        "p (k s) -> p s k", s=NSEQ
                    )
                    nc.scalar.copy(ab4[:, 0, :, t0 : t0 + cnt], src)
                if tf % resc == resc - 1:
                    ep = tf // resc
                    cps = prs.tile([1, NSEQ], F32, tag="rs")
                    nc.tensor.matmul(cps[:], onc[:], af[:])
                    rcs = sp.tile([1, NSEQ], F16, tag="rc")
                    with nc.allow_low_precision(reason="fp16 rescale factor is applied and logged identically"):
                        nc.vector.reciprocal(rcs[:], cps[:])
                    nc.scalar.copy(rcst[:, ep * NSEQ : (ep + 1) * NSEQ], rcs[:])
                    if tf != length - 1:
                        bcp = prs.tile([Q, NSEQ], F32, tag="rs")
                        nc.tensor.matmul(bcp[:], onrh[:], rcs[:])
                        sa = sp.tile([Q, NSEQ], F16, tag="af", bufs=3)
                        nc.vector.tensor_mul(sa[:], af[:], bcp[:])
                        af = sa
                if tb % resc == 0 and tb > 0:
                    cpb = prs.tile([1, NSEQ], F32, tag="rs")
                    nc.tensor.matmul(cpb[:], onc[:], ab4[:, 1, :, tb])
                    rp = tb // resc
                    nc.scalar.copy(cbst[:, rp * NSEQ : (rp + 1) * NSEQ], cpb[:])
                    rcb = sp.tile([1, NSEQ], F16, tag="rc")
                    with nc.allow_low_precision(reason="fp16 rescale factor is applied and logged identically"):
                        nc.vector.reciprocal(rcb[:], cpb[:])
                    bcb = prs.tile([Q, NSEQ], F32, tag="rs")
                    nc.tensor.matmul(bcb[:], onrh[:], rcb[:])
                    sb = sp.tile([Q, NSEQ], F16, tag="sa")
                    nc.vector.tensor_mul(sb[:], ab4[:, 1, :, tb], bcb[:])

            # ---- z = psi * btilde into EB, chunked per (epoch, seq) and
            # ordered middle-out: those chunks' inputs complete while the
            # recursion is still running, so they overlap it.
            zorder = []
            lo, hi = (nep - 1) // 2, (nep + 2 - 1) // 2
            if nep == 1:
                zorder = [0]
            else:
                while lo >= 0:
                    zorder.append(lo)
                    if hi < nep:
                        zorder.append(hi)
                    lo, hi = lo - 1, hi + 1
            for ep in zorder:
                for sq in range(NSEQ):
                    r0 = ep * resc
                    nc.gpsimd.tensor_mul(ab4[:, 0, sq, r0 : r0 + resc],
                                         ab4[:, 0, sq, r0 : r0 + resc],
                                         ab4[:, 1, sq, r0 : r0 + resc])

            # ---- epoch normalizers:
            # P[nep-1] = rca[nep-1];  P[ep] = rca[ep] * cb[ep+1] * P[ep+1]
            # kb = broadcast(P / SH_INIT) over partitions
            nc.vector.tensor_copy(ksuf[:, (nep - 1) * NSEQ : nep * NSEQ],
                                  rcst[:, (nep - 1) * NSEQ : nep * NSEQ])
            for ep in range(nep - 2, -1, -1):
                nc.vector.tensor_mul(
                    ksuf[:, ep * NSEQ : (ep + 1) * NSEQ],
                    rcst[:, ep * NSEQ : (ep + 1) * NSEQ],
                    cbst[:, (ep + 1) * NSEQ : (ep + 2) * NSEQ],
                )
                nc.vector.tensor_mul(
                    ksuf[:, ep * NSEQ : (ep + 1) * NSEQ],
                    ksuf[:, ep * NSEQ : (ep + 1) * NSEQ],
                    ksuf[:, (ep + 1) * NSEQ : (ep + 2) * NSEQ],
                )
            kbp = prs.tile([Q, nep * NSEQ], F32, tag="rs")
            nc.tensor.matmul(kbp[:], onrf[:], ksuf[:])
            nc.scalar.copy(kb[:], kbp[:])

            # ---- transpose + log + store ----
            tchunk = min(resc, length)
            for sq in range(NSEQ):
                for ep in range(length // tchunk):
                    tp = pbig.tile([Q, tchunk], F16, tag="big")
                    for u in range(tchunk // Q):
                        t0 = ep * tchunk + u * Q
                        nc.tensor.transpose(
                            tp[:, u * Q : (u + 1) * Q],
                            ab4[:, 0, sq, t0 : t0 + Q],
                            idn16[:],
                        )
                    st = stp.tile([Q, tchunk], F32, tag="st")
                    epp = min(ep, nep - 1)
                    nc.scalar.activation(
                        st[:], tp[:], mybir.ActivationFunctionType.Ln,
                        scale=kb[:, epp * NSEQ + sq : epp * NSEQ + sq + 1],
                    )
                    dst = post_d.ap()[sq, ep * tchunk : (ep + 1) * tchunk, :]
                    nc.sync.dma_start(
                        dst.rearrange("(u p) q -> p u q", p=Q),
                        st[:].rearrange("p (u q) -> p u q", q=Q),
                    )

            nc.sync.dma_start(rcv_d.ap()[:], rcst[:])

    return nc


def _host_prep(transition_logits, init_logits, emission_logits):
    def softmax(v, ax):
        v = v - v.max(axis=ax, keepdims=True)
        ev = np.exp(v)
        return ev / ev.sum(axis=ax, keepdims=True)

    A = softmax(transition_logits.astype(np.float64), 2)
    pi = softmax(init_logits.astype(np.float64), 1)
    Bm = softmax(emission_logits.astype(np.float64), 2)
    return (A.astype(np.float32), pi.astype(np.float32),
            (2.0 * Bm).astype(np.float32))


_CACHED = {}
_last_in_maps = None


def kernel(inputs, transition_logits, init_logits, emission_logits):
    x = np.ascontiguousarray(np.asarray(inputs, dtype=np.float32))
    A, pi, B2 = _host_prep(np.asarray(transition_logits),
                           np.asarray(init_logits),
                           np.asarray(emission_logits))

    if "nc" not in _CACHED:
        _CACHED["nc"] = build_program(L)
    nc = _CACHED["nc"]

    ident = np.eye(Q, dtype=np.float32)
    onesc = np.ones((Q, 1), np.float16)
    onesrh = np.full((1, Q), SH, np.float16)
    onesrf = np.full((1, Q), 1.0 / SH_INIT, np.float32)

    in_maps = []
    for core in range(N_CORES):
        m, h = core // 2, core % 2
        in_maps.append({
            "obs": np.ascontiguousarray(x[m, NSEQ * h : NSEQ * (h + 1)]),
            "afwd": np.ascontiguousarray(A[m].astype(np.float16)),
            "abwd": np.ascontiguousarray(A[m].T.astype(np.float16)),
            "b2t": np.ascontiguousarray(B2[m].T),
            "pi": np.ascontiguousarray((SH_INIT * pi[m])[:, None].astype(np.float32)),
            "onesc": onesc,
            "onesrh": onesrh,
            "onesrf": onesrf,
            "ident": ident,
            "ident16": ident.astype(np.float16),
        })

    global _last_in_maps
    _last_in_maps = in_maps
    res = run_bass_kernel_spmd(nc, in_maps, list(range(N_CORES)))

    posterior = np.empty((NUM_MODEL, B, L, Q), np.float32)
    loglik = np.empty((NUM_MODEL, B), np.float32)
    for core in range(N_CORES):
        m, h = core // 2, core % 2
        posterior[m, NSEQ * h : NSEQ * (h + 1)] = res.results[core]["post"]
        rc = res.results[core]["rcv"].reshape(NEP, NSEQ).astype(np.float64)
        ll = (-np.log(rc).sum(axis=0) - (NEP - 1) * np.log(SH)
              - np.log(SH_INIT) - L * np.log(2.0))
        loglik[m, NSEQ * h : NSEQ * (h + 1)] = ll.astype(np.float32)
    return posterior, loglik
